# revision 16
# baseline (speedup 1.0000x reference)
"""Trainium2 Bass kernel for nn_ProGraphConv (hyperbolic GNN message passing).

Contract: kernel(**inputs) takes the FULL unsharded inputs and returns the FULL
[30000, 256] float32 output.  Internally the 30000 nodes are sharded across 8
NeuronCores (3750 real + 90 pad slots per core, 30 blocks of 128 slots); edges
are partitioned by destination shard; source tangent features are exchanged
with a piecewise AllGather (fp16); per-destination-block aggregation is done as
a dma_gather + one-hot-indicator matmul (segment sum on the PE).

Mathematical structure exploited (K = 1/c = 1 for both curvatures):
  - ProLinear + hyperbolic bias + the leading logmap0 of ProAgg collapse to
      xt = P1 * mu + P2 * ub        (per node, per sub-manifold)
    where mu = logmap0(x) @ W_ext, ub = logmap0(exp-bias point) is constant,
    and P1/P2 are per-(node, submanifold) scalars given in closed form by the
    Minkowski-norm-preservation of parallel transport (mink_dot(pt,pt) =
    ||ub_sp||^2 exactly).
  - The expmap0/logmap0 round trip between ProAgg and ProAct is the identity,
    so  out = expmap0(clamp(segment_sum, 0, 1000)).
Validated against the jax reference: Frobenius rel err ~4e-5 with fp16
message exchange.
"""

import sys

sys.path.insert(0, "/opt/trn_rl_repo")

import math

import numpy as np

import concourse.bass as bass
import concourse.bacc as bacc
import concourse.mybir as mybir
import concourse.tile as tile
from concourse.bass_utils import run_bass_kernel_spmd

F32 = mybir.dt.float32
F16 = mybir.dt.float16
I16 = mybir.dt.int16

AF = mybir.ActivationFunctionType
OP = mybir.AluOpType

N = 30000
NC = 8
NPC = N // NC          # 3750 real nodes per core
T = 30                 # blocks (tiles) of 128 slots per core
NSH = T * 128          # 3840 slots per core
NPIECE = 5             # AllGather pieces (must divide T)
TPP = T // NPIECE      # tiles per piece
SPP = TPP * 128        # slots per piece (960)
NTAB = NC * NSH        # 30720 rows in the gathered table
P, D = 4, 64
F = 256
MIN = np.float32(1e-15)
EPS = np.float32(1e-7)
MAXN = np.float32(1000.0)
GCH = 8                # chunks (128 idxs) per dma_gather: 1024 idxs = 65 descs

_cache = {}


# ----------------------------------------------------------------- host math
def _host_consts(bias, weight):
    b = np.asarray(bias, dtype=np.float32).reshape(P, D).copy()
    b[:, 0] = 0.0  # proj_tan0
    n = np.maximum(np.linalg.norm(b[:, 1:], axis=-1, keepdims=True), MIN).astype(np.float32)
    space = (np.sinh(n) * b[:, 1:] / n).astype(np.float32)
    time = np.sqrt(1.0 + np.sum(space * space, -1, keepdims=True)).astype(np.float32)
    nn_ = np.maximum(np.linalg.norm(space, axis=-1, keepdims=True), MIN).astype(np.float32)
    th = np.maximum(time, 1.0 + EPS).astype(np.float32)
    arco = np.log(th + np.sqrt(np.maximum(th * th - 1.0, MIN))).astype(np.float32)
    ub = np.concatenate([np.zeros_like(th), arco * space / nn_], -1).astype(np.float32)

    Bn2 = np.sum(ub[:, 1:].astype(np.float64) ** 2, -1).astype(np.float32)  # [P]
    th2 = np.maximum(np.minimum(np.sqrt(np.maximum(Bn2, EPS)), MAXN), MIN).astype(np.float32)
    c2h = np.cosh(th2).astype(np.float32)
    s2h = (np.sinh(th2) / th2).astype(np.float32)

    Wt = np.asarray(weight, dtype=np.float32).T.copy()
    tcols = np.arange(P) * D
    Wt[tcols, :] = 0.0
    Wt[:, tcols] = 0.0
    UbBlk = np.zeros((F, P), dtype=np.float32)
    for p in range(P):
        UbBlk[p * D : (p + 1) * D, p] = ub[p]
    Wext = np.concatenate([Wt, Wt @ UbBlk], axis=1).astype(np.float32)  # [256, 260]
    return ub, Bn2, c2h, s2h, Wext


def _assign_blocks(deg_local):
    """LPT assignment of NPC nodes to T blocks (cap 128), balancing edge counts."""
    import heapq

    order = np.argsort(-deg_local, kind="stable")
    slot = np.full(NPC, -1, dtype=np.int64)
    heap = [(0, 0, b) for b in range(T)]
    heapq.heapify(heap)
    for nl in order:
        spill = []
        while True:
            load, cnt, b = heapq.heappop(heap)
            if cnt < 128:
                break
            spill.append((load, cnt, b))
        for it in spill:
            heapq.heappush(heap, it)
        slot[nl] = b * 128 + cnt
        heapq.heappush(heap, (load + int(deg_local[nl]), cnt + 1, b))
    return slot


def _prepare(x, adj_val, weight, bias, adj_row, adj_col):
    x = np.asarray(x, dtype=np.float32)
    adj_val = np.asarray(adj_val, dtype=np.float32)
    row = np.asarray(adj_row).astype(np.int64)
    col = np.asarray(adj_col).astype(np.int64)

    ub, Bn2, c2h, s2h, Wext = _host_consts(bias, weight)

    ecore = row // NPC
    lrow = row % NPC
    slots = np.empty((NC, NPC), dtype=np.int64)
    deg = np.bincount(row, minlength=N)
    for c in range(NC):
        slots[c] = _assign_blocks(deg[c * NPC : (c + 1) * NPC])

    # table row of global node n: AllGather piece-major layout
    # row = piece*NC*SPP + core*SPP + (slot - piece*SPP)
    own = np.arange(N) // NPC
    sl = slots[own, np.arange(N) % NPC]
    pc = sl // SPP
    tabrow = (pc * NC * SPP + own * SPP + (sl - pc * SPP)).astype(np.int64)

    eslot = slots[ecore, lrow]
    eblk = eslot >> 7
    eseg = (eslot & 127).astype(np.float32)
    egidx = tabrow[col]
    per_core = []
    cpb = 1
    for c in range(NC):
        m = np.nonzero(ecore == c)[0]
        o = m[np.argsort(eblk[m], kind="stable")]
        cnts = np.bincount(eblk[o], minlength=T)
        cpb = max(cpb, int(math.ceil(cnts.max() / 128.0)) if len(o) else 1)
        per_core.append((o, cnts))

    CPB = cpb
    L = CPB * 128

    in_maps = []
    pad_row = np.zeros(F, dtype=np.float32)
    pad_row[0:F:D] = 1.0

    w_in = Wext.reshape(2, 128, 260).transpose(1, 0, 2).copy()
    ubrep_in = np.tile(ub.reshape(1, F), (128, 1)).astype(np.float32)
    ident_in = np.eye(128, dtype=np.float32)
    iota_in = np.tile(np.arange(128, dtype=np.float16)[None, :], (128, 1)).copy()
    c2hb_in = np.tile(c2h[None, :], (128, T)).astype(np.float32)
    s2hb_in = np.tile(s2h[None, :], (128, T)).astype(np.float32)
    s2h2b_in = (2.0 * s2hb_in).astype(np.float32)
    cbb_in = np.tile((s2h * s2h * Bn2)[None, :], (128, T)).astype(np.float32)

    node_of_slot = np.full((NC, NSH), -1, dtype=np.int64)
    for c in range(NC):
        nodes = np.arange(c * NPC, (c + 1) * NPC)
        node_of_slot[c, slots[c]] = nodes

        xsh = np.tile(pad_row, (NSH, 1))
        xsh[slots[c]] = x[nodes]
        x_in = xsh.reshape(T, 128, F).transpose(1, 0, 2).copy()

        o, cnts = per_core[c]
        eidx_np = np.zeros((T, L), dtype=np.int16)
        eseg_np = np.zeros((T, L), dtype=np.float32)
        eval_np = np.zeros((T, L), dtype=np.float32)
        off = 0
        for b in range(T):
            cnt = int(cnts[b])
            sel = o[off : off + cnt]
            off += cnt
            eidx_np[b, :cnt] = egidx[sel].astype(np.int16)
            eseg_np[b, :cnt] = eseg[sel]
            eval_np[b, :cnt] = adj_val[sel]

        idx_in = np.tile(
            eidx_np.reshape(T, CPB * 8, 16).transpose(2, 0, 1), (8, 1, 1)
        ).copy()
        seg_in = eseg_np.reshape(T, CPB, 128).transpose(2, 0, 1).reshape(128, T * CPB).copy()
        val_in = eval_np.reshape(T, CPB, 128).transpose(2, 0, 1).reshape(128, T * CPB).copy()

        in_maps.append(
            dict(
                x_in=x_in, idx_in=idx_in, seg_in=seg_in, val_in=val_in,
                w_in=w_in, ubrep_in=ubrep_in, ident_in=ident_in, iota_in=iota_in,
                c2hb_in=c2hb_in, s2hb_in=s2hb_in, s2h2b_in=s2h2b_in, cbb_in=cbb_in,
            )
        )

    return in_maps, node_of_slot, CPB


# ------------------------------------------------------------ device program
def _build(CPB):
    L = CPB * 128
    nc = bacc.Bacc("TRN2", target_bir_lowering=False, debug=False, num_devices=NC)

    x_in = nc.dram_tensor("x_in", [128, T, F], F32, kind="ExternalInput")
    idx_in = nc.dram_tensor("idx_in", [128, T, CPB * 8], I16, kind="ExternalInput")
    seg_in = nc.dram_tensor("seg_in", [128, T * CPB], F32, kind="ExternalInput")
    val_in = nc.dram_tensor("val_in", [128, T * CPB], F32, kind="ExternalInput")
    w_in = nc.dram_tensor("w_in", [128, 2, 260], F32, kind="ExternalInput")
    ubrep_in = nc.dram_tensor("ubrep_in", [128, F], F32, kind="ExternalInput")
    ident_in = nc.dram_tensor("ident_in", [128, 128], F32, kind="ExternalInput")
    iota_in = nc.dram_tensor("iota_in", [128, 128], F16, kind="ExternalInput")
    c2hb_in = nc.dram_tensor("c2hb_in", [128, T * P], F32, kind="ExternalInput")
    s2hb_in = nc.dram_tensor("s2hb_in", [128, T * P], F32, kind="ExternalInput")
    s2h2b_in = nc.dram_tensor("s2h2b_in", [128, T * P], F32, kind="ExternalInput")
    cbb_in = nc.dram_tensor("cbb_in", [128, T * P], F32, kind="ExternalInput")

    out_sh = nc.dram_tensor("out_sh", [NSH, F], F32, kind="ExternalOutput")
    xt_own = nc.dram_tensor("xt_own", [NSH, F], F16)
    xt_tab = nc.dram_tensor("xt_tab", [NTAB, F], F16, addr_space="Shared")

    TP = T * P

    with tile.TileContext(nc) as tc:
        with (
            tc.tile_pool(name="pers", bufs=1) as pers,
            tc.tile_pool(name="sc", bufs=1) as sc,
        ):
            # persistent loads
            w_sb = pers.tile([128, 2, 260], F32)
            nc.sync.dma_start(out=w_sb[:], in_=w_in[:])
            ubrep = pers.tile([128, F], F32)
            nc.sync.dma_start(out=ubrep[:], in_=ubrep_in[:])
            ident = pers.tile([128, 128], F32)
            nc.sync.dma_start(out=ident[:], in_=ident_in[:])
            iota = pers.tile([128, 128], F16)
            nc.sync.dma_start(out=iota[:], in_=iota_in[:])
            c2hb = pers.tile([128, TP], F32)
            nc.sync.dma_start(out=c2hb[:], in_=c2hb_in[:])
            s2hb = pers.tile([128, TP], F32)
            nc.sync.dma_start(out=s2hb[:], in_=s2hb_in[:])
            s2h2b = pers.tile([128, TP], F32)
            nc.sync.dma_start(out=s2h2b[:], in_=s2h2b_in[:])
            cbb = pers.tile([128, TP], F32)
            nc.sync.dma_start(out=cbb[:], in_=cbb_in[:])
            idx_sb = pers.tile([128, T, CPB * 8], I16)
            nc.sync.dma_start(out=idx_sb[:], in_=idx_in[:])
            seg_sb = pers.tile([128, T * CPB], F32)
            nc.sync.dma_start(out=seg_sb[:], in_=seg_in[:])
            val_sb = pers.tile([128, T * CPB], F32)
            nc.sync.dma_start(out=val_sb[:], in_=val_in[:])

            x_all = pers.tile([128, T, F], F32)
            for pc_ in range(NPIECE):
                s = pc_ * TPP
                nc.sync.dma_start(
                    out=x_all[:, s : s + TPP, :], in_=x_in[:, s : s + TPP, :]
                )
            big2 = pers.tile([128, T, F], F32)
            mu_all = pers.tile([128, T, F], F16)

            def st(name):
                return sc.tile([128, TP], F32, tag=name, name=name)

            # ---------------- Phase A sweep 1 ----------------
            S1 = st("S1")
            x0 = st("x0")
            for t in range(T):
                nc.scalar.activation(big2[:, t, :], x_all[:, t, :], AF.Square)
                b2v = big2[:, t, :].rearrange("p (b d) -> p b d", d=D)
                nc.vector.tensor_reduce(
                    S1[:, t * P : (t + 1) * P], b2v[:, :, 1:D],
                    axis=mybir.AxisListType.X, op=OP.add,
                )
                nc.vector.tensor_copy(x0[:, t * P : (t + 1) * P], x_all[:, t, 0:F:D])

            # chain 1
            n1 = st("n1")
            nc.scalar.activation(n1[:], S1[:], AF.Sqrt)
            nc.vector.tensor_scalar_max(n1[:], n1[:], float(MIN))
            rn1 = st("rn1")
            nc.vector.reciprocal(rn1[:], n1[:])
            th1 = st("th1")
            nc.vector.tensor_scalar_max(th1[:], x0[:], float(1.0 + EPS))
            q2 = st("q2")
            nc.scalar.activation(q2[:], th1[:], AF.Square)
            nc.vector.tensor_scalar(q2[:], q2[:], -1.0, float(MIN), op0=OP.add, op1=OP.max)
            nc.scalar.activation(q2[:], q2[:], AF.Sqrt)
            lg = st("lg")
            nc.vector.tensor_tensor(lg[:], th1[:], q2[:], op=OP.add)
            arco1 = st("arco1")
            nc.scalar.activation(arco1[:], lg[:], AF.Ln)
            scale1 = st("scale1")
            nc.vector.tensor_tensor(scale1[:], arco1[:], rn1[:], op=OP.mult)

            R2 = st("R2")
            anum = st("anum")

            # ---------------- Phase A sweep 2 ----------------
            with (
                tc.tile_pool(name="swp", bufs=3) as swp,
                tc.tile_pool(name="uTp", bufs=4) as uTp,
                tc.tile_pool(name="psT", bufs=2, space="PSUM") as psT,
                tc.tile_pool(name="psMU", bufs=2, space="PSUM") as psMU,
            ):
                for t in range(T):
                    u_t = swp.tile([128, F], F32, tag="u_t")
                    for p in range(P):
                        slc = slice(p * D, (p + 1) * D)
                        sccol = scale1[:, t * P + p : t * P + p + 1]
                        if p < 2:
                            nc.vector.tensor_scalar_mul(
                                u_t[:, slc], x_all[:, t, slc], sccol
                            )
                        else:
                            nc.scalar.mul(u_t[:, slc], x_all[:, t, slc], sccol)
                    ps0 = psT.tile([128, 128], F32, tag="ps0")
                    ps1 = psT.tile([128, 128], F32, tag="ps1")
                    nc.tensor.transpose(ps0[:], u_t[:, 0:128], ident[:])
                    nc.tensor.transpose(ps1[:], u_t[:, 128:256], ident[:])
                    uT0 = uTp.tile([128, 128], F32, tag="uT0")
                    uT1 = uTp.tile([128, 128], F32, tag="uT1")
                    nc.vector.tensor_copy(uT0[:], ps0[:])
                    nc.vector.tensor_copy(uT1[:], ps1[:])
                    mm = psMU.tile([128, 260], F32, tag="mm")
                    nc.tensor.matmul(mm[:], uT0[:], w_sb[:, 0, :], start=True, stop=False)
                    nc.tensor.matmul(mm[:], uT1[:], w_sb[:, 1, :], start=False, stop=True)
                    mu2 = swp.tile([128, F], F32, tag="mu2")
                    nc.scalar.activation(mu2[:], mm[:, 0:F], AF.Square)
                    m2v = mu2[:].rearrange("p (b d) -> p b d", d=D)
                    nc.vector.tensor_reduce(
                        R2[:, t * P : (t + 1) * P], m2v[:],
                        axis=mybir.AxisListType.X, op=OP.add,
                    )
                    nc.vector.tensor_copy(anum[:, t * P : (t + 1) * P], mm[:, F : F + P])
                    nc.vector.tensor_copy(mu_all[:, t, :], mm[:, 0:F])

            # ---------------- chain 2 ----------------
            n2 = st("n2")
            nc.scalar.activation(n2[:], R2[:], AF.Sqrt)
            nc.vector.tensor_scalar_max(n2[:], n2[:], float(MIN))
            rn2 = st("rn2")
            nc.vector.reciprocal(rn2[:], n2[:])
            e = st("e")
            nc.scalar.activation(e[:], n2[:], AF.Exp)
            ie = st("ie")
            nc.vector.reciprocal(ie[:], e[:])
            diff = st("diff")
            nc.vector.tensor_tensor(diff[:], e[:], ie[:], op=OP.subtract)
            s_ = st("s_")
            nc.vector.scalar_tensor_tensor(s_[:], diff[:], 0.5, rn2[:], op0=OP.mult, op1=OP.mult)
            sume = st("sume")
            nc.vector.tensor_tensor(sume[:], e[:], ie[:], op=OP.add)
            cm1 = st("cm1")
            nc.vector.tensor_scalar(cm1[:], sume[:], 0.5, -1.0, op0=OP.mult, op1=OP.add)
            alpha = st("alpha")
            nc.vector.tensor_tensor(alpha[:], anum[:], rn2[:], op=OP.mult)
            g = st("g")
            nc.vector.tensor_tensor(g[:], alpha[:], cm1[:], op=OP.mult)
            nc.vector.tensor_tensor(g[:], g[:], rn2[:], op=OP.mult)
            A_ = st("A_")
            t1 = st("t1")
            nc.vector.tensor_tensor(t1[:], s_[:], c2hb[:], op=OP.mult)
            nc.vector.tensor_tensor(A_[:], g[:], s2hb[:], op=OP.mult)
            nc.vector.tensor_tensor(A_[:], A_[:], t1[:], op=OP.add)
            rsq = st("rsq")
            nc.vector.tensor_tensor(t1[:], A_[:], A_[:], op=OP.mult)
            nc.vector.tensor_tensor(t1[:], t1[:], R2[:], op=OP.mult)
            nc.vector.tensor_tensor(rsq[:], A_[:], anum[:], op=OP.mult)
            nc.vector.tensor_tensor(rsq[:], rsq[:], s2h2b[:], op=OP.mult)
            nc.vector.tensor_tensor(rsq[:], rsq[:], t1[:], op=OP.add)
            nc.vector.tensor_tensor(rsq[:], rsq[:], cbb[:], op=OP.add)
            rt = st("rt")
            nc.scalar.activation(rt[:], rsq[:], AF.Sqrt, bias=1.0)
            n3 = st("n3")
            nc.scalar.activation(n3[:], rsq[:], AF.Sqrt)
            nc.vector.tensor_scalar_max(n3[:], n3[:], float(MIN))
            rn3 = st("rn3")
            nc.vector.reciprocal(rn3[:], n3[:])
            th3 = st("th3")
            nc.vector.tensor_scalar_max(th3[:], rt[:], float(1.0 + EPS))
            q3 = st("q3")
            nc.scalar.activation(q3[:], th3[:], AF.Square)
            nc.vector.tensor_scalar(q3[:], q3[:], -1.0, float(MIN), op0=OP.add, op1=OP.max)
            nc.scalar.activation(q3[:], q3[:], AF.Sqrt)
            lg3 = st("lg3")
            nc.vector.tensor_tensor(lg3[:], th3[:], q3[:], op=OP.add)
            arco3 = st("arco3")
            nc.scalar.activation(arco3[:], lg3[:], AF.Ln)
            Fc = st("Fc")
            nc.vector.tensor_tensor(Fc[:], arco3[:], rn3[:], op=OP.mult)
            P1 = st("P1")
            nc.vector.tensor_tensor(P1[:], Fc[:], A_[:], op=OP.mult)
            P2 = st("P2")
            nc.vector.tensor_tensor(P2[:], Fc[:], s2hb[:], op=OP.mult)

            # ---------------- sweep 3 + piecewise AllGather ----------------
            with tc.tile_pool(name="swp3", bufs=3) as swp3:
                for pc_ in range(NPIECE):
                    for t in range(pc_ * TPP, (pc_ + 1) * TPP):
                        tmp = swp3.tile([128, F], F32, tag="tmp")
                        xt_t = swp3.tile([128, F], F16, tag="xt_t")
                        for p in range(P):
                            slc = slice(p * D, (p + 1) * D)
                            nc.vector.tensor_scalar_mul(
                                tmp[:, slc], ubrep[:, slc],
                                P2[:, t * P + p : t * P + p + 1],
                            )
                            nc.vector.scalar_tensor_tensor(
                                xt_t[:, slc], mu_all[:, t, slc],
                                P1[:, t * P + p : t * P + p + 1], tmp[:, slc],
                                op0=OP.mult, op1=OP.add,
                            )
                        nc.sync.dma_start(
                            out=xt_own[t * 128 : (t + 1) * 128, :], in_=xt_t[:]
                        )
                    nc.gpsimd.collective_compute(
                        "AllGather",
                        OP.bypass,
                        replica_groups=[list(range(NC))],
                        ins=[xt_own[pc_ * SPP : (pc_ + 1) * SPP, :]],
                        outs=[xt_tab[pc_ * NC * SPP : (pc_ + 1) * NC * SPP, :]],
                    )

            # ---------------- Phase C + D (per block) ----------------
            with (
                tc.tile_pool(name="msgp", bufs=2) as msgp,
                tc.tile_pool(name="stp", bufs=4) as stp,
                tc.tile_pool(name="psC", bufs=2, space="PSUM") as psC,
                tc.tile_pool(name="dp", bufs=2) as dp,
                tc.tile_pool(name="dsm", bufs=2) as dsm,
            ):
                for b in range(T):
                    msg = msgp.tile([128, CPB, F], F16, tag="msg")
                    for off in range(0, CPB, GCH):
                        nch = min(GCH, CPB - off)
                        nc.gpsimd.dma_gather(
                            msg[:, off : off + nch, :], xt_tab[:],
                            idx_sb[:, b, off * 8 : (off + nch) * 8],
                            nch * 128, nch * 128, F,
                        )
                    acc = psC.tile([128, F], F32, tag="acc")
                    for j in range(CPB):
                        c_ = b * CPB + j
                        stj = stp.tile([128, 128], F16, tag="stj")
                        nc.vector.tensor_scalar(
                            stj[:], iota[:],
                            seg_sb[:, c_ : c_ + 1], val_sb[:, c_ : c_ + 1],
                            op0=OP.is_equal, op1=OP.mult,
                        )
                        nc.tensor.matmul(
                            acc[:], stj[:], msg[:, j, :],
                            start=(j == 0), stop=(j == CPB - 1),
                        )
                    # Phase D
                    t6 = dp.tile([128, F], F32, tag="t6")
                    nc.vector.tensor_scalar(
                        t6[:], acc[:], 0.0, float(MAXN), op0=OP.max, op1=OP.min
                    )
                    t62 = dp.tile([128, F], F32, tag="t62")
                    nc.vector.tensor_tensor(t62[:], t6[:], t6[:], op=OP.mult)
                    S6 = dsm.tile([128, P], F32, tag="S6")
                    t62v = t62[:].rearrange("p (b d) -> p b d", d=D)
                    nc.vector.tensor_reduce(
                        S6[:], t62v[:], axis=mybir.AxisListType.X, op=OP.add
                    )
                    n6 = dsm.tile([128, P], F32, tag="n6")
                    nc.scalar.activation(n6[:], S6[:], AF.Sqrt)
                    nc.vector.tensor_scalar_max(n6[:], n6[:], float(MIN))
                    e6 = dsm.tile([128, P], F32, tag="e6")
                    nc.scalar.activation(e6[:], n6[:], AF.Exp)
                    i6 = dsm.tile([128, P], F32, tag="i6")
                    nc.vector.reciprocal(i6[:], e6[:])
                    rn6 = dsm.tile([128, P], F32, tag="rn6")
                    nc.vector.reciprocal(rn6[:], n6[:])
                    c6 = dsm.tile([128, P], F32, tag="c6")
                    nc.vector.tensor_tensor(c6[:], e6[:], i6[:], op=OP.add)
                    nc.vector.tensor_scalar_mul(c6[:], c6[:], 0.5)
                    d6 = dsm.tile([128, P], F32, tag="d6")
                    nc.vector.tensor_tensor(d6[:], e6[:], i6[:], op=OP.subtract)
                    s6n = dsm.tile([128, P], F32, tag="s6n")
                    nc.vector.scalar_tensor_tensor(
                        s6n[:], d6[:], 0.5, rn6[:], op0=OP.mult, op1=OP.mult
                    )
                    outt = dp.tile([128, F], F32, tag="outt")
                    for p in range(P):
                        slc = slice(p * D, (p + 1) * D)
                        nc.vector.tensor_scalar_mul(
                            outt[:, slc], t6[:, slc], s6n[:, p : p + 1]
                        )
                    nc.vector.tensor_copy(outt[:, 0:F:D], c6[:])
                    nc.sync.dma_start(
                        out=out_sh[b * 128 : (b + 1) * 128, :], in_=outt[:]
                    )

    nc.compile()
    return nc


# ------------------------------------------------------------------- driver
def kernel(x, adj_val, weight, bias, adj_row, adj_col, **_unused):
    in_maps, node_of_slot, CPB = _prepare(x, adj_val, weight, bias, adj_row, adj_col)

    key = ("prog", CPB)
    if key not in _cache:
        _cache[key] = _build(CPB)
    nc = _cache[key]

    res = run_bass_kernel_spmd(nc, in_maps, core_ids=list(range(NC)))

    out = np.empty((N, F), dtype=np.float32)
    for c in range(NC):
        o = res.results[c]["out_sh"]
        real = node_of_slot[c] >= 0
        out[node_of_slot[c][real]] = o[real]
    return out
